# revision 7
# baseline (speedup 1.0000x reference)
"""DeepGraphSAGE (4-layer, 128-dim) Trainium2 Bass kernel, 8-way SPMD.

Sharding: nodes are block-partitioned across the 8 cores (6250 each); edges are
partitioned by destination core and sorted by destination node.  Each layer:
  1. every core holds a full replica of the previous layer's node features in
     DRAM (built by AllGather),
  2. gathers x[src] rows for its local edges via batched indirect DMA,
  3. segment-sums them into per-64-node windows with one-hot matmuls on the PE
     (one-hot built on the fly by a DVE is_equal against an iota table),
  4. normalizes by 1/deg, applies the two dense 128x128 matmuls, the folded
     BatchNorm (scales pre-multiplied into the weights on the host), bias,
     ReLU and residual,
  5. transposes back to row-major and AllGathers the new features.
The small 128x128 weights are replicated to every core.
"""

import os
import sys
from dataclasses import dataclass, field

import numpy as np

for _p in ("/opt/trn_rl_repo", "/root/.axon_site/_ro/trn_rl_repo"):
    if os.path.isdir(_p) and _p not in sys.path:
        sys.path.append(_p)

import concourse.bass as bass
import concourse.bacc as bacc
import concourse.mybir as mybir
import concourse.tile as tile
from concourse.masks import make_identity

F32 = mybir.dt.float32
I32 = mybir.dt.int32
RELU = mybir.ActivationFunctionType.Relu
EPS = 1e-5
CH = 128          # edges per chunk (PE contraction dim)
WIN = 64          # segment-sum window width (nodes per PSUM window)


@dataclass
class Cfg:
    N: int = 50000
    E: int = 800000
    D: int = 128
    C: int = 8                      # cores
    batch_chunks: int = 32          # max chunks per indirect gather

    @property
    def NLOC(self):
        return self.N // self.C

    @property
    def NWIN(self):
        return -(-self.NLOC // WIN)


def preprocess(cfg: Cfg, edge_index: np.ndarray):
    """Edge schedule: per-core chunk tables, identical shape across cores."""
    src_all = edge_index[0].astype(np.int64)
    dst_all = edge_index[1].astype(np.int64)
    deg = np.bincount(dst_all, minlength=cfg.N).astype(np.float32)
    inv_deg = (1.0 / np.maximum(deg, 1.0)).astype(np.float32)

    per_core = []
    core_of = dst_all // cfg.NLOC
    for c in range(cfg.C):
        m = core_of == c
        s = src_all[m]
        d = dst_all[m] - c * cfg.NLOC
        order = np.argsort(d, kind="stable")
        per_core.append((s[order], d[order]))

    # per-window chunk counts (max over cores -> SPMD-identical schedule)
    nch_w = []
    bounds = []  # per core: window edge ranges
    for c in range(cfg.C):
        d = per_core[c][1]
        b = np.searchsorted(d, np.arange(0, cfg.NWIN + 1) * WIN)
        bounds.append(b)
    for w in range(cfg.NWIN):
        mx = 1
        for c in range(cfg.C):
            cnt = bounds[c][w + 1] - bounds[c][w]
            mx = max(mx, -(-int(cnt) // CH))
        nch_w.append(mx)
    T = sum(nch_w)

    src_idx = np.zeros((cfg.C, CH, T), np.int32)
    dst_off = np.full((cfg.C, CH, T), -1.0, np.float32)
    slot0 = np.concatenate([[0], np.cumsum(nch_w)])
    for c in range(cfg.C):
        s, d = per_core[c]
        for w in range(cfg.NWIN):
            lo, hi = bounds[c][w], bounds[c][w + 1]
            es, ed = s[lo:hi], d[lo:hi] - w * WIN
            n = hi - lo
            t0 = slot0[w]
            nch = -(-int(n) // CH) if n else 0
            pad = nch * CH - n
            if pad:
                es = np.concatenate([es, np.zeros(pad, np.int64)])
                ed = np.concatenate([ed, np.full(pad, -1.0)])
            if nch:
                src_idx[c, :, t0:t0 + nch] = es.reshape(nch, CH).T
                dst_off[c, :, t0:t0 + nch] = ed.reshape(nch, CH).T.astype(np.float32)

    # gather batches: consecutive windows, <= batch_chunks chunks each
    batches = []  # (w_start, w_end, t_start, t_end)
    w = 0
    while w < cfg.NWIN:
        w0, t0 = w, slot0[w]
        n = 0
        while w < cfg.NWIN and (n + nch_w[w] <= cfg.batch_chunks or w == w0):
            n += nch_w[w]
            w += 1
        batches.append((w0, w, int(t0), int(slot0[w])))

    maxn = max(nch_w)
    iota_rep = np.tile(np.arange(WIN, dtype=np.float32), (CH, maxn))
    return dict(inv_deg=inv_deg, src_idx=src_idx, dst_off=dst_off,
                nch_w=nch_w, batches=batches, T=T, maxn=maxn, iota_rep=iota_rep)


def fold_weights(Wp, bp, Wl, bl, Wr, bn_gamma, bn_beta, bn_mean, bn_var):
    """Fold BN (eval mode) into the SAGE weights: relu(bn(h)) ==
    relu(agg @ Wl' + x @ Wr' + c)."""
    a = bn_gamma / np.sqrt(bn_var + EPS)           # [4, D]
    b = bn_beta - bn_mean * a                      # [4, D]
    Wl_f = (Wl * a[:, None, :]).astype(np.float32)  # scale output cols
    Wr_f = (Wr * a[:, None, :]).astype(np.float32)
    c_f = (bl * a + b).astype(np.float32)          # [4, D]
    return Wl_f, Wr_f, c_f


def build_program(cfg: Cfg, nch_w, batches, T, maxn):
    NLOC, NWIN, D, C = cfg.NLOC, cfg.NWIN, cfg.D, cfg.C
    n_dense = -(-NLOC // 512)
    n_tp = -(-NLOC // 128)
    slot0 = np.concatenate([[0], np.cumsum(nch_w)])
    rg = [list(range(C))]

    nc = bacc.Bacc("TRN2", target_bir_lowering=False, debug=False, num_devices=C)

    x_in = nc.dram_tensor("x_slice", [NLOC, D], F32, kind="ExternalInput")
    src_in = nc.dram_tensor("src_idx", [CH, T], I32, kind="ExternalInput")
    dstoff_in = nc.dram_tensor("dst_off", [CH, T], F32, kind="ExternalInput")
    iota_in = nc.dram_tensor("iota_rep", [CH, maxn * WIN], F32, kind="ExternalInput")
    invdeg_in = nc.dram_tensor("inv_deg_sl", [1, NLOC], F32, kind="ExternalInput")
    w_in = nc.dram_tensor("weights", [9, D, D], F32, kind="ExternalInput")
    b_in = nc.dram_tensor("biases", [9, D], F32, kind="ExternalInput")
    y_out = nc.dram_tensor("y", [NLOC, D], F32, kind="ExternalOutput")

    with tile.TileContext(nc) as tc:
        with (
            tc.tile_pool(name="consts", bufs=1) as cs,
            tc.tile_pool(name="work", bufs=2) as wk,
            tc.tile_pool(name="big", bufs=1) as bg,
            tc.tile_pool(name="psum", bufs=2, space="PSUM") as ps,
            tc.tile_pool(name="dram", bufs=1, space="DRAM") as dr,
        ):
            # ---------- persistent loads ----------
            src_sb = cs.tile([CH, T], I32)
            dst_sb = cs.tile([CH, T], F32)
            iota_sb = cs.tile([CH, maxn * WIN], F32)
            invdeg_sl = cs.tile([1, NLOC], F32)
            w_sb = cs.tile([128, 9 * D], F32)
            b_sb = cs.tile([128, 9], F32)
            ident = cs.tile([128, 128], F32)
            make_identity(nc, ident[:])
            nc.sync.dma_start(out=src_sb[:], in_=src_in[:])
            nc.sync.dma_start(out=dst_sb[:], in_=dstoff_in[:])
            nc.sync.dma_start(out=iota_sb[:], in_=iota_in[:])
            nc.sync.dma_start(out=invdeg_sl[:], in_=invdeg_in[:])
            nc.sync.dma_start(
                out=w_sb[:].rearrange("p (k d) -> p k d", d=D),
                in_=w_in[:].rearrange("k p d -> p k d"),
            )
            nc.sync.dma_start(out=b_sb[:], in_=b_in[:].rearrange("k p -> p k"))

            def W(k):
                return w_sb[:, k * D:(k + 1) * D]

            def B(k):
                return b_sb[:, k:k + 1]

            # inv_deg broadcast across partitions via K=1 outer product
            ones_sb = cs.tile([1, 128], F32)
            nc.vector.memset(ones_sb[:], 1.0)
            invdeg_bc = cs.tile([128, NLOC], F32)
            for gidx in range(n_dense):
                lo, hi = gidx * 512, min((gidx + 1) * 512, NLOC)
                ps_bc = ps.tile([128, 512], F32, space="PSUM", tag="dense")
                nc.tensor.matmul(out=ps_bc[:, :hi - lo], lhsT=ones_sb[:],
                                 rhs=invdeg_sl[:, lo:hi], start=True, stop=True)
                nc.vector.tensor_copy(out=invdeg_bc[:, lo:hi],
                                      in_=ps_bc[:, :hi - lo])

            stage = bg.tile([128, n_tp * 128], F32)   # row-major staging

            def transpose_in(dst_T, src_dram):
                """DRAM rows [NLOC, D] -> feature-major SBUF [D, NLOC]."""
                for i in range(n_tp):
                    lo, hi = i * 128, min((i + 1) * 128, NLOC)
                    p = hi - lo
                    xr = wk.tile([128, D], F32, tag="xr")
                    nc.sync.dma_start(out=xr[:p], in_=src_dram[lo:hi, :])
                    ps_t = ps.tile([128, 128], F32, space="PSUM", tag="tp")
                    nc.tensor.transpose(out=ps_t[:, :p], in_=xr[:p],
                                        identity=ident[:p, :p])
                    nc.vector.tensor_copy(out=dst_T[:, lo:hi], in_=ps_t[:, :p])

            def transpose_out(src_T):
                """Feature-major SBUF [D, NLOC] -> row-major stage tile."""
                for i in range(n_tp):
                    lo, hi = i * 128, min((i + 1) * 128, NLOC)
                    p = hi - lo
                    ps_t = ps.tile([128, 128], F32, space="PSUM", tag="tp")
                    nc.tensor.transpose(out=ps_t[:p, :], in_=src_T[:, lo:hi],
                                        identity=ident[:, :])
                    nc.vector.tensor_copy(out=stage[:p, i * 128:(i + 1) * 128],
                                          in_=ps_t[:p, :])

            def stage_to_dram_safe(dst_dram, nrows):
                """stage [128, n_tp*128] (tile-transposed rows) -> row-major."""
                if nrows % 128 == 0:
                    nc.sync.dma_start(
                        out=dst_dram[:nrows, :].rearrange("(t p) d -> p t d",
                                                          p=128),
                        in_=stage[:, :nrows].rearrange("p (t d) -> p t d", d=D),
                    )
                else:
                    for i in range(n_tp):
                        lo, hi = i * 128, min((i + 1) * 128, nrows)
                        p = hi - lo
                        nc.sync.dma_start(out=dst_dram[lo:hi, :],
                                          in_=stage[:p, i * 128:(i + 1) * 128])

            # ---------- prologue: load + project ----------
            xT = bg.tile([128, NLOC], F32, tag="xT", bufs=2)
            transpose_in(xT, x_in)

            def dense_layer(out_T, rhs1_T, w1, rhs2_T, w2, bias_col, residual_T):
                """out = relu(rhs1 @ w1 + rhs2 @ w2 + bias) (+ residual)."""
                for gidx in range(n_dense):
                    lo, hi = gidx * 512, min((gidx + 1) * 512, NLOC)
                    n = hi - lo
                    ps_h = ps.tile([128, 512], F32, space="PSUM", tag="dense")
                    nc.tensor.matmul(out=ps_h[:, :n], lhsT=w1, rhs=rhs1_T[:, lo:hi],
                                     start=True, stop=(rhs2_T is None))
                    if rhs2_T is not None:
                        nc.tensor.matmul(out=ps_h[:, :n], lhsT=w2,
                                         rhs=rhs2_T[:, lo:hi], start=False, stop=True)
                    if residual_T is None:
                        nc.scalar.activation(out=out_T[:, lo:hi], in_=ps_h[:, :n],
                                             func=RELU, bias=bias_col, scale=1.0)
                    else:
                        h_sb = wk.tile([128, 512], F32, tag="hsb")
                        nc.scalar.activation(out=h_sb[:, :n], in_=ps_h[:, :n],
                                             func=RELU, bias=bias_col, scale=1.0)
                        nc.vector.tensor_add(out=out_T[:, lo:hi],
                                             in0=residual_T[:, lo:hi],
                                             in1=h_sb[:, :n])

            xT1 = bg.tile([128, NLOC], F32, tag="xT", bufs=2)
            dense_layer(xT1, xT, W(0), None, None, B(0), None)
            cur_xT = xT1

            # ---------- 4 SAGE layers ----------
            aggT = bg.tile([128, NLOC], F32, tag="aggT", bufs=1)
            for layer in range(4):
                # publish current features to the replicated table
                transpose_out(cur_xT)
                cc_in = dr.tile([n_tp * 128, D], F32, tag="cc_in")
                stage_to_dram_safe(cc_in, n_tp * 128)
                xtab = dr.tile([cfg.N, D], F32, tag="xtab", addr_space="Shared",
                               bufs=2)
                nc.gpsimd.collective_compute(
                    "AllGather", mybir.AluOpType.bypass, replica_groups=rg,
                    ins=[cc_in[:NLOC, :].opt()], outs=[xtab.opt()],
                )

                # segment-sum into aggT (one [128,1]-offset gather per chunk;
                # multi-index indirect DMA is miscompiled on HW)
                for w in range(NWIN):
                    nw = nch_w[w]
                    t0 = int(slot0[w])
                    lo = w * WIN
                    hi = min(lo + WIN, NLOC)
                    onehot = wk.tile([128, maxn * WIN], F32, tag="onehot",
                                     bufs=4)
                    nc.vector.tensor_tensor(
                        out=onehot[:, :nw * WIN].rearrange(
                            "p (c w) -> p c w", w=WIN),
                        in0=dst_sb[:, t0:t0 + nw].to_broadcast(
                            [128, nw, WIN]),
                        in1=iota_sb[:, :nw * WIN].rearrange(
                            "p (c w) -> p c w", w=WIN),
                        op=mybir.AluOpType.is_equal,
                    )
                    ps_w = ps.tile([128, WIN], F32, space="PSUM", tag="seg")
                    for k in range(nw):
                        g_sb = wk.tile([128, D], F32, tag="g", bufs=24)
                        nc.gpsimd.indirect_dma_start(
                            out=g_sb[:],
                            out_offset=None,
                            in_=xtab[:],
                            in_offset=bass.IndirectOffsetOnAxis(
                                ap=src_sb[:, t0 + k:t0 + k + 1], axis=0),
                        )
                        nc.tensor.matmul(
                            out=ps_w[:],
                            lhsT=g_sb[:],
                            rhs=onehot[:, k * WIN:(k + 1) * WIN],
                            start=(k == 0), stop=(k == nw - 1),
                        )
                    nc.vector.tensor_mul(out=aggT[:, lo:hi],
                                         in0=ps_w[:, :hi - lo],
                                         in1=invdeg_bc[:, lo:hi])

                # dense part
                new_xT = bg.tile([128, NLOC], F32, tag="xT", bufs=2)
                dense_layer(new_xT, aggT, W(1 + layer), cur_xT, W(5 + layer),
                            B(1 + layer), cur_xT if layer < 3 else None)
                cur_xT = new_xT

            # ---------- epilogue ----------
            transpose_out(cur_xT)
            stage_to_dram_safe(y_out, NLOC)

    nc.compile()
    return nc


@dataclass
class Built:
    cfg: Cfg
    nc: object
    sched: dict


_built_cache: dict = {}


def build(cfg: Cfg, edge_index: np.ndarray) -> Built:
    sched = preprocess(cfg, edge_index)
    key = (cfg.N, cfg.E, cfg.C, tuple(sched["nch_w"]))
    if key not in _built_cache:
        nc = build_program(cfg, sched["nch_w"], sched["batches"], sched["T"],
                           sched["maxn"])
        _built_cache[key] = nc
    return Built(cfg, _built_cache[key], sched)


def make_in_maps(cfg: Cfg, built: Built, x, Wp, bp, Wl, bl, Wr,
                 bn_gamma, bn_beta, bn_mean, bn_var):
    sched = built.sched
    Wl_f, Wr_f, c_f = fold_weights(Wp, bp, Wl, bl, Wr,
                                   bn_gamma, bn_beta, bn_mean, bn_var)
    weights = np.stack([Wp] + list(Wl_f) + list(Wr_f)).astype(np.float32)
    biases = np.stack([bp] + list(c_f) + [np.zeros_like(bp)] * 4).astype(np.float32)
    in_maps = []
    for c in range(cfg.C):
        lo = c * cfg.NLOC
        in_maps.append({
            "x_slice": np.ascontiguousarray(x[lo:lo + cfg.NLOC]),
            "src_idx": np.ascontiguousarray(sched["src_idx"][c]),
            "dst_off": np.ascontiguousarray(sched["dst_off"][c]),
            "iota_rep": sched["iota_rep"],
            "inv_deg_sl": sched["inv_deg"][None, lo:lo + cfg.NLOC],
            "weights": weights,
            "biases": biases,
        })
    return in_maps


def kernel(x, edge_index, Wp, bp, Wl, bl, Wr, bn_gamma, bn_beta, bn_mean,
           bn_var) -> np.ndarray:
    from concourse.bass_utils import run_bass_kernel_spmd

    cfg = Cfg()
    x = np.asarray(x, np.float32)
    edge_index = np.asarray(edge_index)
    built = build(cfg, edge_index)
    in_maps = make_in_maps(cfg, built, x, np.asarray(Wp), np.asarray(bp),
                           np.asarray(Wl), np.asarray(bl), np.asarray(Wr),
                           np.asarray(bn_gamma), np.asarray(bn_beta),
                           np.asarray(bn_mean), np.asarray(bn_var))
    res = run_bass_kernel_spmd(built.nc, in_maps, core_ids=list(range(cfg.C)))
    out = np.concatenate([res.results[c]["y"] for c in range(cfg.C)], axis=0)
    return out.astype(np.float32)


# revision 11
# speedup vs baseline: 19.3495x; 19.3495x over previous
"""DeepGraphSAGE (4-layer, 128-dim) Trainium2 Bass kernel, 8-way SPMD.

Sharding: nodes are block-partitioned across the 8 cores (6250 each); edges are
partitioned by destination core and sorted by destination node.  Each layer:
  1. every core holds a full replica of the previous layer's node features in
     DRAM (built by AllGather),
  2. gathers x[src] rows for its local edges via batched indirect DMA,
  3. segment-sums them into per-64-node windows with one-hot matmuls on the PE
     (one-hot built on the fly by a DVE is_equal against an iota table),
  4. normalizes by 1/deg, applies the two dense 128x128 matmuls, the folded
     BatchNorm (scales pre-multiplied into the weights on the host), bias,
     ReLU and residual,
  5. transposes back to row-major and AllGathers the new features.
The small 128x128 weights are replicated to every core.
"""

import os
import sys
from dataclasses import dataclass, field

import numpy as np

for _p in ("/opt/trn_rl_repo", "/root/.axon_site/_ro/trn_rl_repo"):
    if os.path.isdir(_p) and _p not in sys.path:
        sys.path.append(_p)

import concourse.bass as bass
import concourse.bacc as bacc
import concourse.mybir as mybir
import concourse.tile as tile
from concourse.masks import make_identity

F32 = mybir.dt.float32
I32 = mybir.dt.int32
RELU = mybir.ActivationFunctionType.Relu
EPS = 1e-5
CH = 128          # edges per chunk (PE contraction dim)
WIN = 64          # segment-sum window width (nodes per PSUM window)


@dataclass
class Cfg:
    N: int = 50000
    E: int = 800000
    D: int = 128
    C: int = 8                      # cores
    batch_chunks: int = 32          # max chunks per indirect gather

    @property
    def NLOC(self):
        return self.N // self.C

    @property
    def NWIN(self):
        return -(-self.NLOC // WIN)


def preprocess(cfg: Cfg, edge_index: np.ndarray):
    """Edge schedule: per-core chunk tables, identical shape across cores."""
    src_all = edge_index[0].astype(np.int64)
    dst_all = edge_index[1].astype(np.int64)
    deg = np.bincount(dst_all, minlength=cfg.N).astype(np.float32)
    inv_deg = (1.0 / np.maximum(deg, 1.0)).astype(np.float32)

    per_core = []
    core_of = dst_all // cfg.NLOC
    for c in range(cfg.C):
        m = core_of == c
        s = src_all[m]
        d = dst_all[m] - c * cfg.NLOC
        order = np.argsort(d, kind="stable")
        per_core.append((s[order], d[order]))

    # per-window chunk counts (max over cores -> SPMD-identical schedule)
    nch_w = []
    bounds = []  # per core: window edge ranges
    for c in range(cfg.C):
        d = per_core[c][1]
        b = np.searchsorted(d, np.arange(0, cfg.NWIN + 1) * WIN)
        bounds.append(b)
    for w in range(cfg.NWIN):
        mx = 1
        for c in range(cfg.C):
            cnt = bounds[c][w + 1] - bounds[c][w]
            mx = max(mx, -(-int(cnt) // CH))
        nch_w.append(mx)
    T = sum(nch_w)

    src_idx = np.zeros((cfg.C, CH, T), np.int32)
    dst_off = np.full((cfg.C, CH, T), -1.0, np.float32)
    slot0 = np.concatenate([[0], np.cumsum(nch_w)])
    for c in range(cfg.C):
        s, d = per_core[c]
        for w in range(cfg.NWIN):
            lo, hi = bounds[c][w], bounds[c][w + 1]
            es, ed = s[lo:hi], d[lo:hi] - w * WIN
            n = hi - lo
            t0 = slot0[w]
            nch = -(-int(n) // CH) if n else 0
            pad = nch * CH - n
            if pad:
                es = np.concatenate([es, np.zeros(pad, np.int64)])
                ed = np.concatenate([ed, np.full(pad, -1.0)])
            if nch:
                src_idx[c, :, t0:t0 + nch] = es.reshape(nch, CH).T
                dst_off[c, :, t0:t0 + nch] = ed.reshape(nch, CH).T.astype(np.float32)

    # gather batches: consecutive windows, <= batch_chunks chunks each
    batches = []  # (w_start, w_end, t_start, t_end)
    w = 0
    while w < cfg.NWIN:
        w0, t0 = w, slot0[w]
        n = 0
        while w < cfg.NWIN and (n + nch_w[w] <= cfg.batch_chunks or w == w0):
            n += nch_w[w]
            w += 1
        batches.append((w0, w, int(t0), int(slot0[w])))

    maxn = max(nch_w)
    iota_rep = np.tile(np.arange(WIN, dtype=np.float32), (CH, maxn))
    return dict(inv_deg=inv_deg, src_idx=src_idx, dst_off=dst_off,
                nch_w=nch_w, batches=batches, T=T, maxn=maxn, iota_rep=iota_rep)


def fold_weights(Wp, bp, Wl, bl, Wr, bn_gamma, bn_beta, bn_mean, bn_var):
    """Fold BN (eval mode) into the SAGE weights: relu(bn(h)) ==
    relu(agg @ Wl' + x @ Wr' + c)."""
    a = bn_gamma / np.sqrt(bn_var + EPS)           # [4, D]
    b = bn_beta - bn_mean * a                      # [4, D]
    Wl_f = (Wl * a[:, None, :]).astype(np.float32)  # scale output cols
    Wr_f = (Wr * a[:, None, :]).astype(np.float32)
    c_f = (bl * a + b).astype(np.float32)          # [4, D]
    return Wl_f, Wr_f, c_f


def build_program(cfg: Cfg, nch_w, batches, T, maxn, ablate=frozenset()):
    NLOC, NWIN, D, C = cfg.NLOC, cfg.NWIN, cfg.D, cfg.C
    n_dense = -(-NLOC // 512)
    n_tp = -(-NLOC // 128)
    slot0 = np.concatenate([[0], np.cumsum(nch_w)])
    rg = [list(range(C))]

    nc = bacc.Bacc("TRN2", target_bir_lowering=False, debug=False, num_devices=C)

    x_in = nc.dram_tensor("x_slice", [NLOC, D], F32, kind="ExternalInput")
    src_in = nc.dram_tensor("src_idx", [CH, T], I32, kind="ExternalInput")
    dstoff_in = nc.dram_tensor("dst_off", [CH, T], F32, kind="ExternalInput")
    iota_in = nc.dram_tensor("iota_rep", [CH, maxn * WIN], F32, kind="ExternalInput")
    invdeg_in = nc.dram_tensor("inv_deg_sl", [1, NLOC], F32, kind="ExternalInput")
    w_in = nc.dram_tensor("weights", [9, D, D], F32, kind="ExternalInput")
    b_in = nc.dram_tensor("biases", [9, D], F32, kind="ExternalInput")
    y_out = nc.dram_tensor("y", [NLOC, D], F32, kind="ExternalOutput")

    if "bare" in ablate:
        with tile.TileContext(nc) as tc:
            with tc.tile_pool(name="sb", bufs=2) as sb:
                t = sb.tile([128, D], F32)
                nc.sync.dma_start(out=t[:], in_=x_in[:128, :])
                nc.sync.dma_start(out=y_out[:128, :], in_=t[:])
        nc.compile()
        return nc

    with tile.TileContext(nc) as tc:
        with (
            tc.tile_pool(name="consts", bufs=1) as cs,
            tc.tile_pool(name="work", bufs=2) as wk,
            tc.tile_pool(name="big", bufs=1) as bg,
            tc.tile_pool(name="psum", bufs=2, space="PSUM") as ps,
            tc.tile_pool(name="dram", bufs=1, space="DRAM") as dr,
        ):
            # ---------- persistent loads ----------
            src_sb = cs.tile([CH, T], I32)
            dst_sb = cs.tile([CH, T], F32)
            iota_sb = cs.tile([CH, maxn * WIN], F32)
            invdeg_sl = cs.tile([1, NLOC], F32)
            w_sb = cs.tile([128, 9 * D], F32)
            b_sb = cs.tile([128, 9], F32)
            ident = cs.tile([128, 128], F32)
            make_identity(nc, ident[:])
            nc.sync.dma_start(out=src_sb[:], in_=src_in[:])
            nc.sync.dma_start(out=dst_sb[:], in_=dstoff_in[:])
            nc.sync.dma_start(out=iota_sb[:], in_=iota_in[:])
            nc.sync.dma_start(out=invdeg_sl[:], in_=invdeg_in[:])
            nc.sync.dma_start(
                out=w_sb[:].rearrange("p (k d) -> p k d", d=D),
                in_=w_in[:].rearrange("k p d -> p k d"),
            )
            nc.sync.dma_start(out=b_sb[:], in_=b_in[:].rearrange("k p -> p k"))

            def W(k):
                return w_sb[:, k * D:(k + 1) * D]

            def B(k):
                return b_sb[:, k:k + 1]

            # inv_deg broadcast across partitions via K=1 outer product
            ones_sb = cs.tile([1, 128], F32)
            nc.vector.memset(ones_sb[:], 1.0)
            invdeg_bc = cs.tile([128, NLOC], F32)
            for gidx in range(n_dense):
                lo, hi = gidx * 512, min((gidx + 1) * 512, NLOC)
                ps_bc = ps.tile([128, 512], F32, space="PSUM", tag="dense")
                nc.tensor.matmul(out=ps_bc[:, :hi - lo], lhsT=ones_sb[:],
                                 rhs=invdeg_sl[:, lo:hi], start=True, stop=True)
                nc.vector.tensor_copy(out=invdeg_bc[:, lo:hi],
                                      in_=ps_bc[:, :hi - lo])

            stage = bg.tile([128, n_tp * 128], F32)   # row-major staging

            def transpose_in(dst_T, src_dram):
                """DRAM rows [NLOC, D] -> feature-major SBUF [D, NLOC]."""
                for i in range(n_tp):
                    lo, hi = i * 128, min((i + 1) * 128, NLOC)
                    p = hi - lo
                    xr = wk.tile([128, D], F32, tag="xr")
                    nc.sync.dma_start(out=xr[:p], in_=src_dram[lo:hi, :])
                    ps_t = ps.tile([128, 128], F32, space="PSUM", tag="tp")
                    nc.tensor.transpose(out=ps_t[:, :p], in_=xr[:p],
                                        identity=ident[:p, :p])
                    nc.vector.tensor_copy(out=dst_T[:, lo:hi], in_=ps_t[:, :p])

            def transpose_out(src_T):
                """Feature-major SBUF [D, NLOC] -> row-major stage tile."""
                for i in range(n_tp):
                    lo, hi = i * 128, min((i + 1) * 128, NLOC)
                    p = hi - lo
                    ps_t = ps.tile([128, 128], F32, space="PSUM", tag="tp")
                    nc.tensor.transpose(out=ps_t[:p, :], in_=src_T[:, lo:hi],
                                        identity=ident[:, :])
                    nc.vector.tensor_copy(out=stage[:p, i * 128:(i + 1) * 128],
                                          in_=ps_t[:p, :])

            def stage_to_dram_safe(dst_dram, nrows):
                """stage [128, n_tp*128] (tile-transposed rows) -> row-major."""
                if nrows % 128 == 0:
                    nc.sync.dma_start(
                        out=dst_dram[:nrows, :].rearrange("(t p) d -> p t d",
                                                          p=128),
                        in_=stage[:, :nrows].rearrange("p (t d) -> p t d", d=D),
                    )
                else:
                    for i in range(n_tp):
                        lo, hi = i * 128, min((i + 1) * 128, nrows)
                        p = hi - lo
                        nc.sync.dma_start(out=dst_dram[lo:hi, :],
                                          in_=stage[:p, i * 128:(i + 1) * 128])

            # ---------- prologue: load + project ----------
            xT = bg.tile([128, NLOC], F32, tag="xT", bufs=2)
            transpose_in(xT, x_in)

            def dense_layer(out_T, rhs1_T, w1, rhs2_T, w2, bias_col, residual_T):
                """out = relu(rhs1 @ w1 + rhs2 @ w2 + bias) (+ residual)."""
                for gidx in range(n_dense):
                    lo, hi = gidx * 512, min((gidx + 1) * 512, NLOC)
                    n = hi - lo
                    ps_h = ps.tile([128, 512], F32, space="PSUM", tag="dense")
                    nc.tensor.matmul(out=ps_h[:, :n], lhsT=w1, rhs=rhs1_T[:, lo:hi],
                                     start=True, stop=(rhs2_T is None))
                    if rhs2_T is not None:
                        nc.tensor.matmul(out=ps_h[:, :n], lhsT=w2,
                                         rhs=rhs2_T[:, lo:hi], start=False, stop=True)
                    if residual_T is None:
                        nc.scalar.activation(out=out_T[:, lo:hi], in_=ps_h[:, :n],
                                             func=RELU, bias=bias_col, scale=1.0)
                    else:
                        h_sb = wk.tile([128, 512], F32, tag="hsb")
                        nc.scalar.activation(out=h_sb[:, :n], in_=ps_h[:, :n],
                                             func=RELU, bias=bias_col, scale=1.0)
                        nc.vector.tensor_add(out=out_T[:, lo:hi],
                                             in0=residual_T[:, lo:hi],
                                             in1=h_sb[:, :n])

            xT1 = bg.tile([128, NLOC], F32, tag="xT", bufs=2)
            dense_layer(xT1, xT, W(0), None, None, B(0), None)
            cur_xT = xT1

            # ---------- 4 SAGE layers ----------
            aggT = bg.tile([128, NLOC], F32, tag="aggT", bufs=1)
            for layer in range(4):
                # publish current features to the replicated table
                transpose_out(cur_xT)
                cc_in = dr.tile([n_tp * 128, D], F32, tag="cc_in")
                stage_to_dram_safe(cc_in, n_tp * 128)
                xtab = dr.tile([cfg.N, D], F32, tag="xtab", addr_space="Shared",
                               bufs=2)
                if "ag" not in ablate:
                    nc.gpsimd.collective_compute(
                        "AllGather", mybir.AluOpType.bypass, replica_groups=rg,
                        ins=[cc_in[:NLOC, :].opt()], outs=[xtab.opt()],
                    )
                elif layer == 0:
                    nc.sync.dma_start(out=xtab[:NLOC, :], in_=cc_in[:NLOC, :])

                # segment-sum into aggT (one [128,1]-offset gather per chunk;
                # multi-index indirect DMA is miscompiled on HW)
                if "seg" in ablate and layer == 0:
                    nc.vector.memset(aggT[:], 0.0)
                if "gather" in ablate and layer == 0:
                    g_fix = bg.tile([128, D], F32, tag="gfix")
                    nc.vector.memset(g_fix[:], 0.0)
                for w in range(NWIN if "seg" not in ablate else 0):
                    nw = nch_w[w]
                    t0 = int(slot0[w])
                    lo = w * WIN
                    hi = min(lo + WIN, NLOC)
                    onehot = wk.tile([128, maxn * WIN], F32, tag="onehot",
                                     bufs=4)
                    nc.vector.tensor_tensor(
                        out=onehot[:, :nw * WIN].rearrange(
                            "p (c w) -> p c w", w=WIN),
                        in0=dst_sb[:, t0:t0 + nw].to_broadcast(
                            [128, nw, WIN]),
                        in1=iota_sb[:, :nw * WIN].rearrange(
                            "p (c w) -> p c w", w=WIN),
                        op=mybir.AluOpType.is_equal,
                    )
                    ps_w = ps.tile([128, WIN], F32, space="PSUM", tag="seg")
                    for k in range(nw):
                        if "gather" not in ablate:
                            g_sb = wk.tile([128, D], F32, tag="g", bufs=40)
                            nc.gpsimd.indirect_dma_start(
                                out=g_sb[:],
                                out_offset=None,
                                in_=xtab[:],
                                in_offset=bass.IndirectOffsetOnAxis(
                                    ap=src_sb[:, t0 + k:t0 + k + 1], axis=0),
                            )
                        else:
                            g_sb = g_fix
                        nc.tensor.matmul(
                            out=ps_w[:],
                            lhsT=g_sb[:],
                            rhs=onehot[:, k * WIN:(k + 1) * WIN],
                            start=(k == 0), stop=(k == nw - 1),
                        )
                    nc.vector.tensor_mul(out=aggT[:, lo:hi],
                                         in0=ps_w[:, :hi - lo],
                                         in1=invdeg_bc[:, lo:hi])

                # dense part
                new_xT = bg.tile([128, NLOC], F32, tag="xT", bufs=2)
                dense_layer(new_xT, aggT, W(1 + layer), cur_xT, W(5 + layer),
                            B(1 + layer), cur_xT if layer < 3 else None)
                cur_xT = new_xT

            # ---------- epilogue ----------
            transpose_out(cur_xT)
            stage_to_dram_safe(y_out, NLOC)

    nc.compile()
    return nc


@dataclass
class Built:
    cfg: Cfg
    nc: object
    sched: dict


_built_cache: dict = {}


def build(cfg: Cfg, edge_index: np.ndarray) -> Built:
    sched = preprocess(cfg, edge_index)
    key = (cfg.N, cfg.E, cfg.C, tuple(sched["nch_w"]))
    if key not in _built_cache:
        nc = build_program(cfg, sched["nch_w"], sched["batches"], sched["T"],
                           sched["maxn"])
        _built_cache[key] = nc
    return Built(cfg, _built_cache[key], sched)


def make_in_maps(cfg: Cfg, built: Built, x, Wp, bp, Wl, bl, Wr,
                 bn_gamma, bn_beta, bn_mean, bn_var):
    sched = built.sched
    Wl_f, Wr_f, c_f = fold_weights(Wp, bp, Wl, bl, Wr,
                                   bn_gamma, bn_beta, bn_mean, bn_var)
    weights = np.stack([Wp] + list(Wl_f) + list(Wr_f)).astype(np.float32)
    biases = np.stack([bp] + list(c_f) + [np.zeros_like(bp)] * 4).astype(np.float32)
    in_maps = []
    for c in range(cfg.C):
        lo = c * cfg.NLOC
        in_maps.append({
            "x_slice": np.ascontiguousarray(x[lo:lo + cfg.NLOC]),
            "src_idx": np.ascontiguousarray(sched["src_idx"][c]),
            "dst_off": np.ascontiguousarray(sched["dst_off"][c]),
            "iota_rep": sched["iota_rep"],
            "inv_deg_sl": sched["inv_deg"][None, lo:lo + cfg.NLOC],
            "weights": weights,
            "biases": biases,
        })
    return in_maps


def kernel(x, edge_index, Wp, bp, Wl, bl, Wr, bn_gamma, bn_beta, bn_mean,
           bn_var) -> np.ndarray:
    from concourse.bass_utils import run_bass_kernel_spmd

    cfg = Cfg()
    x = np.asarray(x, np.float32)
    edge_index = np.asarray(edge_index)
    built = build(cfg, edge_index)
    in_maps = make_in_maps(cfg, built, x, np.asarray(Wp), np.asarray(bp),
                           np.asarray(Wl), np.asarray(bl), np.asarray(Wr),
                           np.asarray(bn_gamma), np.asarray(bn_beta),
                           np.asarray(bn_mean), np.asarray(bn_var))
    res = run_bass_kernel_spmd(built.nc, in_maps, core_ids=list(range(cfg.C)))
    out = np.concatenate([res.results[c]["y"] for c in range(cfg.C)], axis=0)
    return out.astype(np.float32)


# revision 14
# speedup vs baseline: 36.5492x; 1.8889x over previous
"""DeepGraphSAGE (4-layer, 128-dim) Trainium2 Bass kernel, 8-way SPMD.

Sharding: nodes are block-partitioned across the 8 cores (6250 each); edges are
partitioned by destination core and sorted by destination node.  Each layer:
  1. every core holds a full replica of the previous layer's node features in
     DRAM (built by AllGather),
  2. gathers x[src] rows for its local edges via batched indirect DMA,
  3. segment-sums them into per-64-node windows with one-hot matmuls on the PE
     (one-hot built on the fly by a DVE is_equal against an iota table),
  4. normalizes by 1/deg, applies the two dense 128x128 matmuls, the folded
     BatchNorm (scales pre-multiplied into the weights on the host), bias,
     ReLU and residual,
  5. transposes back to row-major and AllGathers the new features.
The small 128x128 weights are replicated to every core.
"""

import os
import sys
from dataclasses import dataclass, field

import numpy as np

for _p in ("/opt/trn_rl_repo", "/root/.axon_site/_ro/trn_rl_repo"):
    if os.path.isdir(_p) and _p not in sys.path:
        sys.path.append(_p)

import concourse.bass as bass
import concourse.bacc as bacc
import concourse.mybir as mybir
import concourse.tile as tile
from concourse.masks import make_identity

F32 = mybir.dt.float32
I32 = mybir.dt.int32
RELU = mybir.ActivationFunctionType.Relu
EPS = 1e-5
CH = 128          # edges per chunk (PE contraction dim)
WIN = 64          # segment-sum window width (nodes per PSUM window)


@dataclass
class Cfg:
    N: int = 50000
    E: int = 800000
    D: int = 128
    C: int = 8                      # cores
    batch_chunks: int = 32          # max chunks per indirect gather

    @property
    def NLOC(self):
        return self.N // self.C

    @property
    def NWIN(self):
        return -(-self.NLOC // WIN)


def _balance_nodes(cfg: Cfg, deg: np.ndarray):
    """Degree-balanced node relabeling: LPT-deal nodes into (core, window)
    buckets so every window's edge count packs 128-edge chunks tightly
    (removes the SPMD max-over-cores padding).  Returns pi (old->new)."""
    import heapq
    NWIN, NLOC = cfg.NWIN, cfg.NLOC
    nbuck = cfg.C * NWIN
    cap = np.array([min(WIN, NLOC - (b % NWIN) * WIN) for b in range(nbuck)],
                   np.int64)
    order = np.argsort(-deg, kind="stable")
    heap = [(0.0, b) for b in range(nbuck)]
    heapq.heapify(heap)
    cnt = np.zeros(nbuck, np.int64)
    esum = np.zeros(nbuck, np.int64)
    pi = np.empty(cfg.N, np.int64)
    base = np.array([(b // NWIN) * NLOC + (b % NWIN) * WIN
                     for b in range(nbuck)], np.int64)
    for v in order:
        while True:
            s, b = heapq.heappop(heap)
            if cnt[b] < cap[b]:
                break
        pi[v] = base[b] + cnt[b]
        cnt[b] += 1
        esum[b] += deg[v]
        if cnt[b] < cap[b]:
            heapq.heappush(heap, (float(esum[b]), b))
    return pi


def preprocess(cfg: Cfg, edge_index: np.ndarray):
    """Edge schedule: per-core chunk tables, identical shape across cores."""
    src_raw = edge_index[0].astype(np.int64)
    dst_raw = edge_index[1].astype(np.int64)
    deg_raw = np.bincount(dst_raw, minlength=cfg.N)
    pi = _balance_nodes(cfg, deg_raw)
    sigma = np.argsort(pi)               # new -> old
    src_all = pi[src_raw]
    dst_all = pi[dst_raw]
    deg = np.bincount(dst_all, minlength=cfg.N).astype(np.float32)
    inv_deg = (1.0 / np.maximum(deg, 1.0)).astype(np.float32)

    per_core = []
    core_of = dst_all // cfg.NLOC
    for c in range(cfg.C):
        m = core_of == c
        s = src_all[m]
        d = dst_all[m] - c * cfg.NLOC
        order = np.argsort(d, kind="stable")
        per_core.append((s[order], d[order]))

    # per-window chunk counts (max over cores -> SPMD-identical schedule)
    nch_w = []
    bounds = []  # per core: window edge ranges
    for c in range(cfg.C):
        d = per_core[c][1]
        b = np.searchsorted(d, np.arange(0, cfg.NWIN + 1) * WIN)
        bounds.append(b)
    for w in range(cfg.NWIN):
        mx = 1
        for c in range(cfg.C):
            cnt = bounds[c][w + 1] - bounds[c][w]
            mx = max(mx, -(-int(cnt) // CH))
        nch_w.append(mx)
    T = sum(nch_w)

    src_idx = np.zeros((cfg.C, CH, T), np.int32)
    dst_off = np.full((cfg.C, CH, T), -1.0, np.float32)
    slot0 = np.concatenate([[0], np.cumsum(nch_w)])
    for c in range(cfg.C):
        s, d = per_core[c]
        for w in range(cfg.NWIN):
            lo, hi = bounds[c][w], bounds[c][w + 1]
            es, ed = s[lo:hi], d[lo:hi] - w * WIN
            n = hi - lo
            t0 = slot0[w]
            nch = -(-int(n) // CH) if n else 0
            pad = nch * CH - n
            if pad:
                es = np.concatenate([es, np.zeros(pad, np.int64)])
                ed = np.concatenate([ed, np.full(pad, -1.0)])
            if nch:
                src_idx[c, :, t0:t0 + nch] = es.reshape(nch, CH).T
                dst_off[c, :, t0:t0 + nch] = ed.reshape(nch, CH).T.astype(np.float32)

    # gather batches: consecutive windows, <= batch_chunks chunks each
    batches = []  # (w_start, w_end, t_start, t_end)
    w = 0
    while w < cfg.NWIN:
        w0, t0 = w, slot0[w]
        n = 0
        while w < cfg.NWIN and (n + nch_w[w] <= cfg.batch_chunks or w == w0):
            n += nch_w[w]
            w += 1
        batches.append((w0, w, int(t0), int(slot0[w])))

    maxn = max(nch_w)
    iota_rep = np.tile(np.arange(WIN, dtype=np.float32), (CH, maxn))
    return dict(inv_deg=inv_deg, src_idx=src_idx, dst_off=dst_off,
                nch_w=nch_w, batches=batches, T=T, maxn=maxn, iota_rep=iota_rep,
                pi=pi, sigma=sigma)


def fold_weights(Wp, bp, Wl, bl, Wr, bn_gamma, bn_beta, bn_mean, bn_var):
    """Fold BN (eval mode) into the SAGE weights: relu(bn(h)) ==
    relu(agg @ Wl' + x @ Wr' + c)."""
    a = bn_gamma / np.sqrt(bn_var + EPS)           # [4, D]
    b = bn_beta - bn_mean * a                      # [4, D]
    Wl_f = (Wl * a[:, None, :]).astype(np.float32)  # scale output cols
    Wr_f = (Wr * a[:, None, :]).astype(np.float32)
    c_f = (bl * a + b).astype(np.float32)          # [4, D]
    return Wl_f, Wr_f, c_f


def build_program(cfg: Cfg, nch_w, batches, T, maxn, ablate=frozenset()):
    NLOC, NWIN, D, C = cfg.NLOC, cfg.NWIN, cfg.D, cfg.C
    n_dense = -(-NLOC // 512)
    n_tp = -(-NLOC // 128)
    slot0 = np.concatenate([[0], np.cumsum(nch_w)])
    rg = [list(range(C))]

    nc = bacc.Bacc("TRN2", target_bir_lowering=False, debug=False, num_devices=C)

    x_in = nc.dram_tensor("x_slice", [NLOC, D], F32, kind="ExternalInput")
    src_in = nc.dram_tensor("src_idx", [CH, T], I32, kind="ExternalInput")
    dstoff_in = nc.dram_tensor("dst_off", [CH, T], F32, kind="ExternalInput")
    iota_in = nc.dram_tensor("iota_rep", [CH, maxn * WIN], F32, kind="ExternalInput")
    invdeg_in = nc.dram_tensor("inv_deg_sl", [1, NLOC], F32, kind="ExternalInput")
    w_in = nc.dram_tensor("weights", [9, D, D], F32, kind="ExternalInput")
    b_in = nc.dram_tensor("biases", [9, D], F32, kind="ExternalInput")
    y_out = nc.dram_tensor("y", [NLOC, D], F32, kind="ExternalOutput")

    if "bare" in ablate:
        with tile.TileContext(nc) as tc:
            with tc.tile_pool(name="sb", bufs=2) as sb:
                t = sb.tile([128, D], F32)
                nc.sync.dma_start(out=t[:], in_=x_in[:128, :])
                nc.sync.dma_start(out=y_out[:128, :], in_=t[:])
        nc.compile()
        return nc

    with tile.TileContext(nc) as tc:
        with (
            tc.tile_pool(name="consts", bufs=1) as cs,
            tc.tile_pool(name="work", bufs=2) as wk,
            tc.tile_pool(name="big", bufs=1) as bg,
            tc.tile_pool(name="psum", bufs=2, space="PSUM") as ps,
            tc.tile_pool(name="dram", bufs=1, space="DRAM") as dr,
        ):
            # ---------- persistent loads ----------
            src_sb = cs.tile([CH, T], I32)
            dst_sb = cs.tile([CH, T], F32)
            iota_sb = cs.tile([CH, maxn * WIN], F32)
            invdeg_sl = cs.tile([1, NLOC], F32)
            w_sb = cs.tile([128, 9 * D], F32)
            b_sb = cs.tile([128, 9], F32)
            ident = cs.tile([128, 128], F32)
            make_identity(nc, ident[:])
            nc.sync.dma_start(out=src_sb[:], in_=src_in[:])
            nc.sync.dma_start(out=dst_sb[:], in_=dstoff_in[:])
            nc.sync.dma_start(out=iota_sb[:], in_=iota_in[:])
            nc.sync.dma_start(out=invdeg_sl[:], in_=invdeg_in[:])
            nc.sync.dma_start(
                out=w_sb[:].rearrange("p (k d) -> p k d", d=D),
                in_=w_in[:].rearrange("k p d -> p k d"),
            )
            nc.sync.dma_start(out=b_sb[:], in_=b_in[:].rearrange("k p -> p k"))

            def W(k):
                return w_sb[:, k * D:(k + 1) * D]

            def B(k):
                return b_sb[:, k:k + 1]

            # inv_deg broadcast across partitions via K=1 outer product
            ones_sb = cs.tile([1, 128], F32)
            nc.vector.memset(ones_sb[:], 1.0)
            invdeg_bc = cs.tile([128, NLOC], F32)
            for gidx in range(n_dense):
                lo, hi = gidx * 512, min((gidx + 1) * 512, NLOC)
                ps_bc = ps.tile([128, 512], F32, space="PSUM", tag="dense")
                nc.tensor.matmul(out=ps_bc[:, :hi - lo], lhsT=ones_sb[:],
                                 rhs=invdeg_sl[:, lo:hi], start=True, stop=True)
                nc.vector.tensor_copy(out=invdeg_bc[:, lo:hi],
                                      in_=ps_bc[:, :hi - lo])

            stage = bg.tile([128, n_tp * 128], F32)   # row-major staging

            def transpose_in(dst_T, src_dram):
                """DRAM rows [NLOC, D] -> feature-major SBUF [D, NLOC]."""
                for i in range(n_tp):
                    lo, hi = i * 128, min((i + 1) * 128, NLOC)
                    p = hi - lo
                    xr = wk.tile([128, D], F32, tag="xr")
                    nc.sync.dma_start(out=xr[:p], in_=src_dram[lo:hi, :])
                    ps_t = ps.tile([128, 128], F32, space="PSUM", tag="tp")
                    nc.tensor.transpose(out=ps_t[:, :p], in_=xr[:p],
                                        identity=ident[:p, :p])
                    nc.vector.tensor_copy(out=dst_T[:, lo:hi], in_=ps_t[:, :p])

            def transpose_out(src_T):
                """Feature-major SBUF [D, NLOC] -> row-major stage tile."""
                for i in range(n_tp):
                    lo, hi = i * 128, min((i + 1) * 128, NLOC)
                    p = hi - lo
                    ps_t = ps.tile([128, 128], F32, space="PSUM", tag="tp")
                    nc.tensor.transpose(out=ps_t[:p, :], in_=src_T[:, lo:hi],
                                        identity=ident[:, :])
                    nc.vector.tensor_copy(out=stage[:p, i * 128:(i + 1) * 128],
                                          in_=ps_t[:p, :])

            def stage_to_dram_safe(dst_dram, nrows):
                """stage [128, n_tp*128] (tile-transposed rows) -> row-major."""
                if nrows % 128 == 0:
                    nc.sync.dma_start(
                        out=dst_dram[:nrows, :].rearrange("(t p) d -> p t d",
                                                          p=128),
                        in_=stage[:, :nrows].rearrange("p (t d) -> p t d", d=D),
                    )
                else:
                    for i in range(n_tp):
                        lo, hi = i * 128, min((i + 1) * 128, nrows)
                        p = hi - lo
                        nc.sync.dma_start(out=dst_dram[lo:hi, :],
                                          in_=stage[:p, i * 128:(i + 1) * 128])

            # ---------- prologue: load + project ----------
            xT = bg.tile([128, NLOC], F32, tag="xT", bufs=2)
            transpose_in(xT, x_in)

            def dense_layer(out_T, rhs1_T, w1, rhs2_T, w2, bias_col, residual_T):
                """out = relu(rhs1 @ w1 + rhs2 @ w2 + bias) (+ residual)."""
                for gidx in range(n_dense):
                    lo, hi = gidx * 512, min((gidx + 1) * 512, NLOC)
                    n = hi - lo
                    ps_h = ps.tile([128, 512], F32, space="PSUM", tag="dense")
                    nc.tensor.matmul(out=ps_h[:, :n], lhsT=w1, rhs=rhs1_T[:, lo:hi],
                                     start=True, stop=(rhs2_T is None))
                    if rhs2_T is not None:
                        nc.tensor.matmul(out=ps_h[:, :n], lhsT=w2,
                                         rhs=rhs2_T[:, lo:hi], start=False, stop=True)
                    if residual_T is None:
                        nc.scalar.activation(out=out_T[:, lo:hi], in_=ps_h[:, :n],
                                             func=RELU, bias=bias_col, scale=1.0)
                    else:
                        h_sb = wk.tile([128, 512], F32, tag="hsb")
                        nc.scalar.activation(out=h_sb[:, :n], in_=ps_h[:, :n],
                                             func=RELU, bias=bias_col, scale=1.0)
                        nc.vector.tensor_add(out=out_T[:, lo:hi],
                                             in0=residual_T[:, lo:hi],
                                             in1=h_sb[:, :n])

            xT1 = bg.tile([128, NLOC], F32, tag="xT", bufs=2)
            dense_layer(xT1, xT, W(0), None, None, B(0), None)
            cur_xT = xT1

            # ---------- 4 SAGE layers ----------
            aggT = bg.tile([128, NLOC], F32, tag="aggT", bufs=1)
            for layer in range(4):
                # publish current features to the replicated table
                transpose_out(cur_xT)
                cc_in = dr.tile([n_tp * 128, D], F32, tag="cc_in")
                stage_to_dram_safe(cc_in, n_tp * 128)
                xtab = dr.tile([cfg.N, D], F32, tag="xtab", addr_space="Shared",
                               bufs=2)
                if "ag" not in ablate:
                    nc.gpsimd.collective_compute(
                        "AllGather", mybir.AluOpType.bypass, replica_groups=rg,
                        ins=[cc_in[:NLOC, :].opt()], outs=[xtab.opt()],
                    )
                elif layer == 0:
                    nc.sync.dma_start(out=xtab[:NLOC, :], in_=cc_in[:NLOC, :])

                # segment-sum into aggT (one [128,1]-offset gather per chunk;
                # multi-index indirect DMA is miscompiled on HW)
                if "seg" in ablate and layer == 0:
                    nc.vector.memset(aggT[:], 0.0)
                if "gather" in ablate and layer == 0:
                    g_fix = bg.tile([128, D], F32, tag="gfix")
                    nc.vector.memset(g_fix[:], 0.0)
                for w in range(NWIN if "seg" not in ablate else 0):
                    nw = nch_w[w]
                    t0 = int(slot0[w])
                    lo = w * WIN
                    hi = min(lo + WIN, NLOC)
                    onehot = wk.tile([128, maxn * WIN], F32, tag="onehot",
                                     bufs=4)
                    nc.vector.tensor_tensor(
                        out=onehot[:, :nw * WIN].rearrange(
                            "p (c w) -> p c w", w=WIN),
                        in0=dst_sb[:, t0:t0 + nw].to_broadcast(
                            [128, nw, WIN]),
                        in1=iota_sb[:, :nw * WIN].rearrange(
                            "p (c w) -> p c w", w=WIN),
                        op=mybir.AluOpType.is_equal,
                    )
                    ps_w = ps.tile([128, WIN], F32, space="PSUM", tag="seg")
                    for k in range(nw):
                        if "gather" not in ablate:
                            g_sb = wk.tile([128, D], F32, tag="g", bufs=40)
                            nc.gpsimd.indirect_dma_start(
                                out=g_sb[:],
                                out_offset=None,
                                in_=xtab[:],
                                in_offset=bass.IndirectOffsetOnAxis(
                                    ap=src_sb[:, t0 + k:t0 + k + 1], axis=0),
                            )
                        else:
                            g_sb = g_fix
                        nc.tensor.matmul(
                            out=ps_w[:],
                            lhsT=g_sb[:],
                            rhs=onehot[:, k * WIN:(k + 1) * WIN],
                            start=(k == 0), stop=(k == nw - 1),
                        )
                    nc.vector.tensor_mul(out=aggT[:, lo:hi],
                                         in0=ps_w[:, :hi - lo],
                                         in1=invdeg_bc[:, lo:hi])

                # dense part
                new_xT = bg.tile([128, NLOC], F32, tag="xT", bufs=2)
                dense_layer(new_xT, aggT, W(1 + layer), cur_xT, W(5 + layer),
                            B(1 + layer), cur_xT if layer < 3 else None)
                cur_xT = new_xT

            # ---------- epilogue ----------
            transpose_out(cur_xT)
            stage_to_dram_safe(y_out, NLOC)

    nc.compile()
    return nc


@dataclass
class Built:
    cfg: Cfg
    nc: object
    sched: dict


_built_cache: dict = {}


def build(cfg: Cfg, edge_index: np.ndarray) -> Built:
    sched = preprocess(cfg, edge_index)
    key = (cfg.N, cfg.E, cfg.C, tuple(sched["nch_w"]))
    if key not in _built_cache:
        nc = build_program(cfg, sched["nch_w"], sched["batches"], sched["T"],
                           sched["maxn"])
        _built_cache[key] = nc
    return Built(cfg, _built_cache[key], sched)


def make_in_maps(cfg: Cfg, built: Built, x, Wp, bp, Wl, bl, Wr,
                 bn_gamma, bn_beta, bn_mean, bn_var):
    sched = built.sched
    Wl_f, Wr_f, c_f = fold_weights(Wp, bp, Wl, bl, Wr,
                                   bn_gamma, bn_beta, bn_mean, bn_var)
    weights = np.stack([Wp] + list(Wl_f) + list(Wr_f)).astype(np.float32)
    biases = np.stack([bp] + list(c_f) + [np.zeros_like(bp)] * 4).astype(np.float32)
    x_perm = np.ascontiguousarray(x[sched["sigma"]])
    invdeg_perm = sched["inv_deg"]  # already in permuted (new-id) order
    in_maps = []
    for c in range(cfg.C):
        lo = c * cfg.NLOC
        in_maps.append({
            "x_slice": np.ascontiguousarray(x_perm[lo:lo + cfg.NLOC]),
            "src_idx": np.ascontiguousarray(sched["src_idx"][c]),
            "dst_off": np.ascontiguousarray(sched["dst_off"][c]),
            "iota_rep": sched["iota_rep"],
            "inv_deg_sl": np.ascontiguousarray(invdeg_perm[None, lo:lo + cfg.NLOC]),
            "weights": weights,
            "biases": biases,
        })
    return in_maps


def kernel(x, edge_index, Wp, bp, Wl, bl, Wr, bn_gamma, bn_beta, bn_mean,
           bn_var) -> np.ndarray:
    from concourse.bass_utils import run_bass_kernel_spmd

    cfg = Cfg()
    x = np.asarray(x, np.float32)
    edge_index = np.asarray(edge_index)
    built = build(cfg, edge_index)
    in_maps = make_in_maps(cfg, built, x, np.asarray(Wp), np.asarray(bp),
                           np.asarray(Wl), np.asarray(bl), np.asarray(Wr),
                           np.asarray(bn_gamma), np.asarray(bn_beta),
                           np.asarray(bn_mean), np.asarray(bn_var))
    res = run_bass_kernel_spmd(built.nc, in_maps, core_ids=list(range(cfg.C)))
    out = np.concatenate([res.results[c]["y"] for c in range(cfg.C)], axis=0)
    return out[built.sched["pi"]].astype(np.float32)
